# revision 13
# baseline (speedup 1.0000x reference)
"""Trainium2 Bass kernel for nn_CustomModel_21019569946955 (pendulum Lyapunov loss).

Data-parallel over 8 NeuronCores: each core processes B/8 = 8192 samples with
replicated MLP weights. Fully fused single-pass fp32->bf16 pipeline:

  W2 and W2^T both live in SBUF as bf16 (8 MB each), so forward and backward
  run chunk-by-chunk with no DRAM spill and no h1 recompute:

  per chunk (N=512 samples, feature-major):
    fwd:  h1 = tanh(W1^T x^T); u1 = 1-h1^2
          h2 = tanh(W2^T h1); [y_pred; V] = W3^T h2; g2 = (1-h2^2)*W3[:,1]
    bwd:  g1 = u1 * (W2 g2); dVdx = W1 g1
  final stage: batch-major pendulum ODE + penalties + partial sums for the
  scalar custom_loss (combined on host: pure data-parallel mean).
"""
import numpy as np
import concourse.bass as bass
import concourse.tile as tile
from concourse import bacc, mybir
from concourse.bass_utils import run_bass_kernel_spmd
from concourse.masks import make_identity

F32 = mybir.dt.float32
BF16 = mybir.dt.bfloat16
FP16 = mybir.dt.float16
AF = mybir.ActivationFunctionType
ALU = mybir.AluOpType

# problem constants (hardcoded from the reference)
G = 9.8
L, I_, MB, MC, AT, AR = 0.3, 2.0, 1.0, 3.0, 0.2, 0.2
C1 = L * MB            # 0.3
C2 = I_ + L * L * MB   # 2.09
C3 = MB + MC           # 4.0
PEN = 10000.0
ALPHA = 0.1
EPS = 1e-7
C1SQ = C1 * C1
C2C3 = C2 * C3

B, H, D = 65536, 2048, 4
NCORES = 8
BC = B // NCORES        # 8192 samples per core
N = 512                 # batch-chunk (moving free dim)
CH = BC // N            # 16 chunks
KT = H // 128           # 16 feature tiles
FB = BC // 128          # 64 samples per partition in the final stage

# fp32 round-to-nearest-int trick + Cody-Waite 2pi for sin/cos range reduction
RC = float(1.5 * 2 ** 23)
INV2PI = float(1.0 / (2.0 * np.pi))
TWOPI_HI = float(np.float32(2.0 * np.pi))
TWOPI_LO = float(2.0 * np.pi - np.float64(np.float32(2.0 * np.pi)))
HALFPI = float(np.pi / 2)

_NC_CACHE = {}


def build():
    nc = bacc.Bacc("TRN2", target_bir_lowering=False, debug=False)

    xd = nc.declare_dram_parameter("x", [BC, D], F32, isOutput=False)
    yd = nc.declare_dram_parameter("y", [BC], F32, isOutput=False)
    W1d = nc.declare_dram_parameter("W1", [D, H], F32, isOutput=False)
    b1d = nc.declare_dram_parameter("b1", [H], F32, isOutput=False)
    W2d = nc.declare_dram_parameter("W2", [H, H], F32, isOutput=False)
    b2d = nc.declare_dram_parameter("b2", [H], F32, isOutput=False)
    W3d = nc.declare_dram_parameter("W3", [H, 2], F32, isOutput=False)
    b3d = nc.declare_dram_parameter("b3", [2], F32, isOutput=False)

    loss_out = nc.declare_dram_parameter("loss_pen", [BC], F32, isOutput=True)
    part_out = nc.declare_dram_parameter("partials", [128, 2], F32, isOutput=True)

    with tile.TileContext(nc) as tc:
        with tc.tile_pool(name="dram", bufs=1, space="DRAM") as dpool:
            yv_d = dpool.tile([2, BC], F32, tag="yv_d", name="yv_d")
            dv_d = dpool.tile([D, BC], F32, tag="dv_d", name="dv_d")

            with tc.tile_pool(name="wpool", bufs=1) as wpool, \
                 tc.tile_pool(name="small", bufs=1) as small:

                # ---- persistent big tiles ----
                w2f = wpool.tile([128, KT, H], BF16, tag="w2f", name="w2f")
                w2b = wpool.tile([128, KT, H], BF16, tag="w2b", name="w2b")
                h1 = wpool.tile([128, KT, N], BF16, tag="h1", name="h1")
                u1 = wpool.tile([128, KT, N], BF16, tag="u1", name="u1")
                g2 = wpool.tile([128, KT, N], BF16, tag="g2", name="g2")

                # ---- small weights / constants ----
                w1sb = small.tile([D, H], BF16, tag="w1sb", name="w1sb")
                w1t = small.tile([128, KT, D], BF16, tag="w1t", name="w1t")
                w3sb = small.tile([128, KT, 2], BF16, tag="w3sb", name="w3sb")
                identb = small.tile([128, 128], BF16, tag="identb", name="identb")
                sel2 = small.tile([128, 2], FP16, tag="sel2", name="sel2")
                sel4 = small.tile([128, 4], FP16, tag="sel4", name="sel4")

                with tc.tile_pool(name="cvt", bufs=1) as cvt:
                    w1f = cvt.tile([D, H], F32, tag="w1f", name="w1f")
                    nc.sync.dma_start(w1f[:], W1d[:, :])
                    nc.vector.tensor_copy(w1sb[:], w1f[:])

                    w1tf = cvt.tile([128, KT, D], F32, tag="w1tf", name="w1tf")
                    for k in range(KT):
                        nc.gpsimd.dma_start(
                            w1tf[:, k],
                            W1d[:, k * 128:(k + 1) * 128].rearrange("d p -> p d"))
                    nc.vector.tensor_copy(w1t[:], w1tf[:])

                    w3f = cvt.tile([128, KT, 2], F32, tag="w3f", name="w3f")
                    nc.sync.dma_start(w3f[:], W3d.rearrange("(k p) j -> p k j", p=128))
                    nc.vector.tensor_copy(w3sb[:], w3f[:])

                    ident_f = cvt.tile([128, 128], F32, tag="ident_f", name="ident_f")
                    make_identity(nc, ident_f[:])
                    nc.vector.tensor_copy(identb[:], ident_f[:])

                    # selector matrices for combining col-group-packed psum
                    # results: sel4[32g+d, d] = 1 (mod-32 identity columns)
                    m32 = cvt.tile([128, 32], F32, tag="m32", name="m32")
                    nc.vector.tensor_add(m32[:], ident_f[:, 0:32],
                                         ident_f[:, 32:64])
                    nc.vector.tensor_add(m32[:], m32[:], ident_f[:, 64:96])
                    nc.vector.tensor_add(m32[:], m32[:], ident_f[:, 96:128])
                    nc.vector.tensor_copy(sel4[:], m32[:, 0:4])
                    nc.vector.tensor_copy(sel2[:], m32[:, 0:2])

                b1c = small.tile([128, KT], F32, tag="b1c", name="b1c")
                nc.sync.dma_start(b1c[:], b1d.rearrange("(k p) -> p k", p=128))
                b2c = small.tile([128, KT], F32, tag="b2c", name="b2c")
                nc.sync.dma_start(b2c[:], b2d.rearrange("(k p) -> p k", p=128))
                b3c = small.tile([2, 1], F32, tag="b3c", name="b3c")
                nc.sync.dma_start(b3c[:], b3d.rearrange("(p o) -> p o", o=1))
                w3c1 = small.tile([128, KT, 1], F32, tag="w3c1", name="w3c1")
                nc.sync.dma_start(
                    w3c1[:], W3d.rearrange("(k p) j -> p k j", p=128)[:, :, 1:2])
                nw3c1 = small.tile([128, KT, 1], F32, tag="nw3c1", name="nw3c1")
                nc.vector.tensor_scalar_mul(nw3c1[:], w3c1[:], -1.0)



                with tc.tile_pool(name="tmp", bufs=2) as tmp, \
                     tc.tile_pool(name="pp", bufs=2, space="PSUM") as pp:

                    # ---- W2 load + convert + transpose (prologue) ----
                    HQ = H // 4
                    with tc.tile_pool(name="w2cv", bufs=2) as cvp:
                        for k in range(KT):
                            for hh in range(4):
                                t = cvp.tile([128, HQ], F32, tag="w2tmp",
                                             name="w2tmp")
                                nc.sync.dma_start(
                                    t[:], W2d[k * 128:(k + 1) * 128,
                                              hh * HQ:(hh + 1) * HQ])
                                nc.vector.tensor_copy(
                                    w2f[:, k, hh * HQ:(hh + 1) * HQ], t[:])
                                # W2^T tiles: w2b[:, m, k*128..] for this quarter
                                for m in range(4 * hh, 4 * hh + 4):
                                    trp = pp.tile([128, 128], BF16, tag="gps",
                                                  name="trp", bufs=2)
                                    nc.tensor.transpose(
                                        trp[:], w2f[:, k, m * 128:(m + 1) * 128],
                                        identb[:])
                                    if m % 2 == 0:
                                        nc.vector.tensor_copy(
                                            w2b[:, m, k * 128:(k + 1) * 128], trp[:])
                                    else:
                                        nc.scalar.activation(
                                            w2b[:, m, k * 128:(k + 1) * 128], trp[:],
                                            AF.Copy, bias=0.0)

                    # ---- prologue: x chunk 0 + h1/u1 chunk 0 ----
                    def load_x(i):
                        xtf = tmp.tile([D, N], F32, tag="xtf", name="xtf", bufs=1)
                        nc.gpsimd.dma_start(
                            xtf[:], xd[i * N:(i + 1) * N, :].rearrange("n d -> d n"))
                        xt = tmp.tile([D, N], BF16, tag="xt", name="xt")
                        nc.vector.tensor_copy(xt[:], xtf[:])
                        return xt

                    def fwd1(m1, xt):
                        # h1/u1 for feature block m1 from xt (chunk's transposed x)
                        hps = pp.tile([128, N], F32, tag="h1ps", name="hps", bufs=2)
                        nc.tensor.matmul(hps[:], w1sb[:, m1 * 128:(m1 + 1) * 128],
                                         xt[:], start=True, stop=True)
                        nc.scalar.activation(h1[:, m1], hps[:], AF.Tanh,
                                             bias=b1c[:, m1:m1 + 1])
                        nc.vector.tensor_mul(u1[:, m1], h1[:, m1], h1[:, m1])
                        nc.vector.tensor_scalar(u1[:, m1], u1[:, m1], -1.0, 1.0,
                                                ALU.mult, ALU.add)

                    xt_cur = load_x(0)
                    for m1 in range(KT):
                        fwd1(m1, xt_cur)

                    # zero-init the packed psum banks once (combine reads all
                    # 128 partitions; unwritten ones must stay finite)
                    yvp = pp.tile([128, N], F32, tag="yvp", name="yvp0", bufs=1)
                    nc.vector.memset(yvp[:], 0.0)
                    dvp = pp.tile([128, N], F32, tag="dvp", name="dvp0", bufs=1)
                    nc.vector.memset(dvp[:], 0.0)

                    # ---- main loop over chunks ----
                    for i in range(CH):
                        # ---- phase A: fwd W2 / W3 / g2 ----
                        yvp = pp.tile([128, N], F32, tag="yvp", name="yvp", bufs=1)
                        h2ts = [None] * KT
                        for m2 in range(KT):
                            ps = pp.tile([128, N], F32, tag="ps", name="ps", bufs=2)
                            for k in range(KT):
                                nc.tensor.matmul(
                                    ps[:], w2f[:, k, m2 * 128:(m2 + 1) * 128],
                                    h1[:, k], start=(k == 0), stop=(k == KT - 1))
                            # delayed yv burst: 4 concurrent col-group matmuls
                            if m2 % 4 == 0 and m2 > 0:
                                q = m2 // 4 - 1
                                for g in range(4):
                                    mm = 4 * q + g
                                    nc.tensor.matmul(
                                        yvp[32 * g:32 * g + 2, :], w3sb[:, mm],
                                        h2ts[mm][:], start=(q == 0), stop=(q == 3),
                                        tile_position=(0, 32 * g))
                            h2t = tmp.tile([128, N], BF16, tag="h2t", name="h2t",
                                           bufs=6)
                            nc.scalar.activation(h2t[:], ps[:], AF.Tanh,
                                                 bias=b2c[:, m2:m2 + 1])
                            nc.vector.tensor_mul(g2[:, m2], h2t[:], h2t[:])
                            nc.vector.tensor_scalar(
                                g2[:, m2], g2[:, m2], nw3c1[:, m2], w3c1[:, m2],
                                ALU.mult, ALU.add)
                            h2ts[m2] = h2t

                        # x for next chunk
                        if i + 1 < CH:
                            xt_cur = load_x(i + 1)

                        # ---- phase C: bwd W2^T / dVdx (+ fwd W1 of chunk i+1) ----
                        dvp = pp.tile([128, N], F32, tag="dvp", name="dvp", bufs=1)
                        g1hs = [None] * KT
                        for m1 in range(KT):
                            gps = pp.tile([128, N], F32, tag="gps", name="gps", bufs=2)
                            korder = (list(range(KT)) if m1 < KT - 1
                                      else list(range(KT - 1, -1, -1)))
                            for j, k2 in enumerate(korder):
                                nc.tensor.matmul(
                                    gps[:], w2b[:, k2, m1 * 128:(m1 + 1) * 128],
                                    g2[:, k2], start=(j == 0), stop=(j == KT - 1))
                            if m1 == 1:
                                # last yv burst (h2t 12..15 ready by now)
                                for g in range(4):
                                    mm = 12 + g
                                    nc.tensor.matmul(
                                        yvp[32 * g:32 * g + 2, :], w3sb[:, mm],
                                        h2ts[mm][:], start=False, stop=True,
                                        tile_position=(0, 32 * g))
                            if m1 == 2:
                                # yv combine: fold 4 col groups via selector matmul
                                yvcp = tmp.tile([128, N], FP16, tag="pcp",
                                                name="yvcp")
                                nc.vector.tensor_copy(yvcp[:], yvp[:])
                                yv2 = pp.tile([2, N], F32, tag="gps", name="yv2",
                                              bufs=2)
                                nc.tensor.matmul(yv2[:], sel2[:], yvcp[:],
                                                 start=True, stop=True)
                                yvt = tmp.tile([2, N], F32, tag="yvt", name="yvt",
                                               bufs=1)
                                nc.vector.tensor_scalar(yvt[:], yv2[:], b3c[:],
                                                        None, ALU.add)
                                nc.sync.dma_start(
                                    yv_d[:, i * N:(i + 1) * N], yvt[:])
                            # delayed dv burst
                            if m1 % 4 == 0 and m1 > 0:
                                q = m1 // 4 - 1
                                for g in range(4):
                                    mm = 4 * q + g
                                    nc.tensor.matmul(
                                        dvp[32 * g:32 * g + 4, :], w1t[:, mm],
                                        g1hs[mm][:], start=(q == 0), stop=False,
                                        tile_position=(0, 32 * g))
                            g1h = tmp.tile([128, N], BF16, tag="g1h", name="g1h",
                                           bufs=6)
                            nc.vector.tensor_mul(g1h[:], gps[:], u1[:, m1])
                            # interleaved fwd W1 for chunk i+1 (overwrites h1/u1[m1]
                            # after their last chunk-i use)
                            if i + 1 < CH:
                                fwd1(m1, xt_cur)
                            g1hs[m1] = g1h
                        # last dv burst + combine
                        for g in range(4):
                            mm = 12 + g
                            nc.tensor.matmul(
                                dvp[32 * g:32 * g + 4, :], w1t[:, mm],
                                g1hs[mm][:], start=False, stop=True,
                                tile_position=(0, 32 * g))
                        dvcp = tmp.tile([128, N], FP16, tag="pcp", name="dvcp")
                        nc.vector.tensor_copy(dvcp[:], dvp[:])
                        dv2 = pp.tile([D, N], F32, tag="gps", name="dv2", bufs=2)
                        nc.tensor.matmul(dv2[:], sel4[:], dvcp[:],
                                         start=True, stop=True)
                        dvt = tmp.tile([D, N], F32, tag="dvt", name="dvt", bufs=1)
                        nc.vector.tensor_copy(dvt[:], dv2[:])
                        nc.sync.dma_start(dv_d[:, i * N:(i + 1) * N], dvt[:])

                # ---- final stage: batch-major per-sample math ----
                with tc.tile_pool(name="fpool", bufs=1) as fpool:
                    _final_stage(nc, tc, fpool, xd, yd, yv_d, dv_d,
                                 loss_out, part_out)

    nc.compile()
    return nc


def _final_stage(nc, tc, fpool, xd, yd, yv_d, dv_d, loss_out, part_out):
    def plane_from_row(dram_row_ap, tag):
        t = fpool.tile([128, FB], F32, tag=tag, name=tag)
        nc.sync.dma_start(t[:], dram_row_ap.rearrange("(p f) -> p f", p=128))
        return t

    ypred = plane_from_row(yv_d[0], "ypred")
    vpl = plane_from_row(yv_d[1], "vpl")
    dv0 = plane_from_row(dv_d[0], "dv0")
    dv1 = plane_from_row(dv_d[1], "dv1")
    dv2 = plane_from_row(dv_d[2], "dv2")
    dv3 = plane_from_row(dv_d[3], "dv3")
    ypl = plane_from_row(yd[:], "ypl")

    xpl = fpool.tile([128, FB, D], F32, tag="xpl", name="xpl")
    nc.sync.dma_start(xpl[:], xd.rearrange("(p f) d -> p f d", p=128))
    x2 = xpl[:, :, 1]
    x3 = xpl[:, :, 2]
    x4 = xpl[:, :, 3]

    zc = fpool.tile([128, 1], F32, tag="zc", name="zc")
    nc.vector.memset(zc[:], 0.0)

    def ftile(tag):
        return fpool.tile([128, FB], F32, tag=tag, name=tag)

    def sin_reduced(src_ap, negate, bias, tag):
        # sin(bias + (negate ? -src : src)), range-reduced mod 2pi
        w = ftile(tag + "w")
        nc.vector.tensor_scalar(w[:], src_ap, -1.0 if negate else 1.0, bias,
                                ALU.mult, ALU.add)
        t = ftile(tag + "t")
        nc.vector.tensor_scalar(t[:], w[:], INV2PI, RC, ALU.mult, ALU.add)
        r = ftile(tag + "r")
        nc.vector.tensor_scalar(r[:], t[:], RC, None, ALU.subtract)
        a = ftile(tag + "a")
        nc.vector.scalar_tensor_tensor(a[:], r[:], -TWOPI_HI, w[:], ALU.mult, ALU.add)
        y_ = ftile(tag + "y")
        nc.vector.scalar_tensor_tensor(y_[:], r[:], -TWOPI_LO, a[:], ALU.mult, ALU.add)
        o = ftile(tag + "o")
        nc.scalar.activation(o[:], y_[:], AF.Sin, bias=zc[:])
        return o

    s = sin_reduced(x3, False, 0.0, "s")
    c = sin_reduced(x3, True, HALFPI, "c")     # cos(x) = sin(pi/2 - x)

    f = ftile("f")
    nc.vector.scalar_tensor_tensor(f[:], x2, -AT, ypred[:], ALU.mult, ALU.add)

    u = ftile("u")
    nc.vector.tensor_mul(u[:], c[:], c[:])
    den = ftile("den")
    nc.vector.tensor_scalar(den[:], u[:], -C1SQ, C2C3, ALU.mult, ALU.add)
    rden = ftile("rden")
    nc.vector.reciprocal(rden[:], den[:])

    cs = ftile("cs")
    nc.vector.tensor_mul(cs[:], c[:], s[:])
    x4sq = ftile("x4sq")
    nc.vector.tensor_mul(x4sq[:], x4, x4)
    cx4 = ftile("cx4")
    nc.vector.tensor_mul(cx4[:], c[:], x4)
    sx4sq = ftile("sx4sq")
    nc.vector.tensor_mul(sx4sq[:], s[:], x4sq[:])
    csx4sq = ftile("csx4sq")
    nc.vector.tensor_mul(csx4sq[:], cs[:], x4sq[:])
    cf = ftile("cf")
    nc.vector.tensor_mul(cf[:], c[:], f[:])

    # x2p = (G*C1^2*c*s + C2*f - AR*C1*c*x4 - C1*C2*s*x4^2) / den
    p1 = ftile("p1")
    nc.vector.tensor_scalar(p1[:], f[:], C2, None, ALU.mult)
    nc.vector.scalar_tensor_tensor(p1[:], cs[:], G * C1SQ, p1[:], ALU.mult, ALU.add)
    nc.vector.scalar_tensor_tensor(p1[:], cx4[:], -AR * C1, p1[:], ALU.mult, ALU.add)
    nc.vector.scalar_tensor_tensor(p1[:], sx4sq[:], -C1 * C2, p1[:], ALU.mult, ALU.add)
    x2p = ftile("x2p")
    nc.vector.tensor_mul(x2p[:], p1[:], rden[:])

    # x4p = (G*C1*C3*s + C1*c*f - AR*C3*x4 - C1^2*c*s*x4^2) / den
    p2 = ftile("p2")
    nc.vector.tensor_scalar(p2[:], s[:], G * C1 * C3, None, ALU.mult)
    nc.vector.scalar_tensor_tensor(p2[:], cf[:], C1, p2[:], ALU.mult, ALU.add)
    nc.vector.scalar_tensor_tensor(p2[:], x4, -AR * C3, p2[:], ALU.mult, ALU.add)
    nc.vector.scalar_tensor_tensor(p2[:], csx4sq[:], -C1SQ, p2[:], ALU.mult, ALU.add)
    x4p = ftile("x4p")
    nc.vector.tensor_mul(x4p[:], p2[:], rden[:])

    # Vdot = dV . [x2, x2p, x4, x4p]
    vd = ftile("vd")
    nc.vector.tensor_mul(vd[:], dv0[:], x2)
    t_ = ftile("vt")
    nc.vector.tensor_mul(t_[:], dv1[:], x2p[:])
    nc.vector.tensor_add(vd[:], vd[:], t_[:])
    nc.vector.tensor_mul(t_[:], dv2[:], x4)
    nc.vector.tensor_add(vd[:], vd[:], t_[:])
    nc.vector.tensor_mul(t_[:], dv3[:], x4p[:])
    nc.vector.tensor_add(vd[:], vd[:], t_[:])

    # penalties: PEN*relu(-V) + PEN*relu(Vdot)
    pen = ftile("pen")
    nc.vector.tensor_scalar(pen[:], vpl[:], 0.0, -PEN, ALU.min, ALU.mult)
    pen2 = ftile("pen2")
    nc.vector.tensor_scalar(pen2[:], vd[:], 0.0, PEN, ALU.max, ALU.mult)
    nc.vector.tensor_add(pen[:], pen[:], pen2[:])
    nc.sync.dma_start(loss_out.rearrange("(p f) -> p f", p=128), pen[:])

    # partial sums for custom_loss: sum(d^2), sum((y - y_pred)^2)
    ypc = ftile("ypc")
    nc.vector.tensor_scalar(ypc[:], ypred[:], EPS, None, ALU.max)
    l1 = ftile("l1")
    nc.scalar.activation(l1[:], ypc[:], AF.Ln, bias=1.0)
    yc = ftile("yc")
    nc.vector.tensor_scalar(yc[:], ypl[:], EPS, None, ALU.max)
    l2 = ftile("l2")
    nc.scalar.activation(l2[:], yc[:], AF.Ln, bias=1.0)
    dd = ftile("dd")
    nc.vector.tensor_sub(dd[:], l1[:], l2[:])
    d2s = fpool.tile([128, 1], F32, tag="d2s", name="d2s")
    dtmp = ftile("dtmp")
    nc.scalar.activation(dtmp[:], dd[:], AF.Square, bias=zc[:], accum_out=d2s[:])
    ee = ftile("ee")
    nc.vector.tensor_sub(ee[:], ypl[:], ypred[:])
    es = fpool.tile([128, 1], F32, tag="es", name="es")
    nc.scalar.activation(dtmp[:], ee[:], AF.Square, bias=zc[:], accum_out=es[:])

    parts = fpool.tile([128, 2], F32, tag="parts", name="parts")
    nc.vector.tensor_copy(parts[:, 0:1], d2s[:])
    nc.vector.tensor_copy(parts[:, 1:2], es[:])
    nc.sync.dma_start(part_out[:, :], parts[:])


def kernel(**inputs):
    x = np.ascontiguousarray(inputs["x"], dtype=np.float32)
    y = np.ascontiguousarray(inputs["y"], dtype=np.float32)
    W1 = np.ascontiguousarray(inputs["W1"], dtype=np.float32)
    b1 = np.ascontiguousarray(inputs["b1"], dtype=np.float32)
    W2 = np.ascontiguousarray(inputs["W2"], dtype=np.float32)
    b2 = np.ascontiguousarray(inputs["b2"], dtype=np.float32)
    W3 = np.ascontiguousarray(inputs["W3"], dtype=np.float32)
    b3 = np.ascontiguousarray(inputs["b3"], dtype=np.float32)

    if "nc" not in _NC_CACHE:
        _NC_CACHE["nc"] = build()
    nc = _NC_CACHE["nc"]

    in_maps = []
    for cid in range(NCORES):
        sl = slice(cid * BC, (cid + 1) * BC)
        in_maps.append({
            "x": x[sl], "y": y[sl],
            "W1": W1, "b1": b1, "W2": W2, "b2": b2, "W3": W3, "b3": b3,
        })
    res = run_bass_kernel_spmd(nc, in_maps, list(range(NCORES)))

    loss = np.concatenate([res.results[c]["loss_pen"] for c in range(NCORES)])
    parts = np.stack([res.results[c]["partials"] for c in range(NCORES)])
    sums = parts.astype(np.float64).sum(axis=(0, 1))
    scalar = ALPHA * sums[0] / B + (1.0 - ALPHA) * sums[1] / B
    return (loss + np.float32(scalar)).astype(np.float32)


# revision 14
# speedup vs baseline: 1.1611x; 1.1611x over previous
"""Trainium2 Bass kernel for nn_CustomModel_21019569946955 (pendulum Lyapunov loss).

Data-parallel over 8 NeuronCores: each core processes B/8 = 8192 samples with
replicated MLP weights. Fully fused single-pass fp32->bf16 pipeline:

  W2 and W2^T both live in SBUF as bf16 (8 MB each), so forward and backward
  run chunk-by-chunk with no DRAM spill and no h1 recompute:

  per chunk (N=512 samples, feature-major):
    fwd:  h1 = tanh(W1^T x^T); u1 = 1-h1^2
          h2 = tanh(W2^T h1); [y_pred; V] = W3^T h2; g2 = (1-h2^2)*W3[:,1]
    bwd:  g1 = u1 * (W2 g2); dVdx = W1 g1
  final stage: batch-major pendulum ODE + penalties + partial sums for the
  scalar custom_loss (combined on host: pure data-parallel mean).
"""
import numpy as np
import concourse.bass as bass
import concourse.tile as tile
from concourse import bacc, mybir
from concourse.bass_utils import run_bass_kernel_spmd
from concourse.masks import make_identity

F32 = mybir.dt.float32
BF16 = mybir.dt.bfloat16
FP16 = mybir.dt.float16
AF = mybir.ActivationFunctionType
ALU = mybir.AluOpType

# problem constants (hardcoded from the reference)
G = 9.8
L, I_, MB, MC, AT, AR = 0.3, 2.0, 1.0, 3.0, 0.2, 0.2
C1 = L * MB            # 0.3
C2 = I_ + L * L * MB   # 2.09
C3 = MB + MC           # 4.0
PEN = 10000.0
ALPHA = 0.1
EPS = 1e-7
C1SQ = C1 * C1
C2C3 = C2 * C3

B, H, D = 65536, 2048, 4
NCORES = 8
BC = B // NCORES        # 8192 samples per core
N = 512                 # batch-chunk (moving free dim)
CH = BC // N            # 16 chunks
KT = H // 128           # 16 feature tiles
FB = BC // 128          # 64 samples per partition in the final stage

# fp32 round-to-nearest-int trick + Cody-Waite 2pi for sin/cos range reduction
RC = float(1.5 * 2 ** 23)
INV2PI = float(1.0 / (2.0 * np.pi))
TWOPI_HI = float(np.float32(2.0 * np.pi))
TWOPI_LO = float(2.0 * np.pi - np.float64(np.float32(2.0 * np.pi)))
HALFPI = float(np.pi / 2)

_NC_CACHE = {}


def build():
    nc = bacc.Bacc("TRN2", target_bir_lowering=False, debug=False)

    xd = nc.declare_dram_parameter("x", [BC, D], F32, isOutput=False)
    yd = nc.declare_dram_parameter("y", [BC], F32, isOutput=False)
    W1d = nc.declare_dram_parameter("W1", [D, H], F32, isOutput=False)
    b1d = nc.declare_dram_parameter("b1", [H], F32, isOutput=False)
    W2d = nc.declare_dram_parameter("W2", [H, H], F32, isOutput=False)
    b2d = nc.declare_dram_parameter("b2", [H], F32, isOutput=False)
    W3d = nc.declare_dram_parameter("W3", [H, 2], F32, isOutput=False)
    b3d = nc.declare_dram_parameter("b3", [2], F32, isOutput=False)

    loss_out = nc.declare_dram_parameter("loss_pen", [BC], F32, isOutput=True)
    part_out = nc.declare_dram_parameter("partials", [128, 2], F32, isOutput=True)

    with tile.TileContext(nc) as tc:
        with tc.tile_pool(name="dram", bufs=1, space="DRAM") as dpool:
            yv_d = dpool.tile([2, BC], F32, tag="yv_d", name="yv_d")
            dv_d = dpool.tile([D, BC], F32, tag="dv_d", name="dv_d")

            with tc.tile_pool(name="wpool", bufs=1) as wpool, \
                 tc.tile_pool(name="small", bufs=1) as small:

                # ---- persistent big tiles ----
                w2f = wpool.tile([128, KT, H], BF16, tag="w2f", name="w2f")
                w2b = wpool.tile([128, KT, H], BF16, tag="w2b", name="w2b")
                h1 = wpool.tile([128, KT, N], BF16, tag="h1", name="h1")
                u1 = wpool.tile([128, KT, N], BF16, tag="u1", name="u1")
                g2 = wpool.tile([128, KT, N], BF16, tag="g2", name="g2")

                # ---- small weights / constants ----
                w1sb = small.tile([D, H], BF16, tag="w1sb", name="w1sb")
                w1t = small.tile([128, KT, D], BF16, tag="w1t", name="w1t")
                w3sb = small.tile([128, KT, 2], BF16, tag="w3sb", name="w3sb")
                identb = small.tile([128, 128], BF16, tag="identb", name="identb")
                sel2 = small.tile([128, 2], FP16, tag="sel2", name="sel2")
                sel4 = small.tile([128, 4], FP16, tag="sel4", name="sel4")

                with tc.tile_pool(name="cvt", bufs=1) as cvt:
                    w1f = cvt.tile([D, H], F32, tag="w1f", name="w1f")
                    nc.sync.dma_start(w1f[:], W1d[:, :])
                    nc.vector.tensor_copy(w1sb[:], w1f[:])

                    w1tf = cvt.tile([128, KT, D], F32, tag="w1tf", name="w1tf")
                    for k in range(KT):
                        nc.gpsimd.dma_start(
                            w1tf[:, k],
                            W1d[:, k * 128:(k + 1) * 128].rearrange("d p -> p d"))
                    nc.vector.tensor_copy(w1t[:], w1tf[:])

                    w3f = cvt.tile([128, KT, 2], F32, tag="w3f", name="w3f")
                    nc.sync.dma_start(w3f[:], W3d.rearrange("(k p) j -> p k j", p=128))
                    nc.vector.tensor_copy(w3sb[:], w3f[:])

                    ident_f = cvt.tile([128, 128], F32, tag="ident_f", name="ident_f")
                    make_identity(nc, ident_f[:])
                    nc.vector.tensor_copy(identb[:], ident_f[:])

                    # selector matrices for combining col-group-packed psum
                    # results: sel4[32g+d, d] = 1 (mod-32 identity columns)
                    m32 = cvt.tile([128, 32], F32, tag="m32", name="m32")
                    nc.vector.tensor_add(m32[:], ident_f[:, 0:32],
                                         ident_f[:, 32:64])
                    nc.vector.tensor_add(m32[:], m32[:], ident_f[:, 64:96])
                    nc.vector.tensor_add(m32[:], m32[:], ident_f[:, 96:128])
                    nc.vector.tensor_copy(sel4[:], m32[:, 0:4])
                    nc.vector.tensor_copy(sel2[:], m32[:, 0:2])

                b1c = small.tile([128, KT], F32, tag="b1c", name="b1c")
                nc.sync.dma_start(b1c[:], b1d.rearrange("(k p) -> p k", p=128))
                b2c = small.tile([128, KT], F32, tag="b2c", name="b2c")
                nc.sync.dma_start(b2c[:], b2d.rearrange("(k p) -> p k", p=128))
                b3c = small.tile([2, 1], F32, tag="b3c", name="b3c")
                nc.sync.dma_start(b3c[:], b3d.rearrange("(p o) -> p o", o=1))
                w3c1 = small.tile([128, KT, 1], F32, tag="w3c1", name="w3c1")
                nc.sync.dma_start(
                    w3c1[:], W3d.rearrange("(k p) j -> p k j", p=128)[:, :, 1:2])
                nw3c1 = small.tile([128, KT, 1], F32, tag="nw3c1", name="nw3c1")
                nc.vector.tensor_scalar_mul(nw3c1[:], w3c1[:], -1.0)



                with tc.tile_pool(name="tmp", bufs=2) as tmp, \
                     tc.tile_pool(name="pp", bufs=2, space="PSUM") as pp:

                    # ---- W2 load + convert + transpose (prologue) ----
                    HQ = H // 4
                    with tc.tile_pool(name="w2cv", bufs=2) as cvp:
                        for k in range(KT):
                            for hh in range(4):
                                t = cvp.tile([128, HQ], F32, tag="w2tmp",
                                             name="w2tmp")
                                nc.sync.dma_start(
                                    t[:], W2d[k * 128:(k + 1) * 128,
                                              hh * HQ:(hh + 1) * HQ])
                                nc.vector.tensor_copy(
                                    w2f[:, k, hh * HQ:(hh + 1) * HQ], t[:])
                                # W2^T tiles: w2b[:, m, k*128..] for this quarter
                                for m in range(4 * hh, 4 * hh + 4):
                                    trp = pp.tile([128, 128], BF16, tag="gps",
                                                  name="trp", bufs=2)
                                    nc.tensor.transpose(
                                        trp[:], w2f[:, k, m * 128:(m + 1) * 128],
                                        identb[:])
                                    if m % 2 == 0:
                                        nc.vector.tensor_copy(
                                            w2b[:, m, k * 128:(k + 1) * 128], trp[:])
                                    else:
                                        nc.scalar.activation(
                                            w2b[:, m, k * 128:(k + 1) * 128], trp[:],
                                            AF.Copy, bias=0.0)

                    # ---- prologue: x chunk 0 + h1/u1 chunk 0 ----
                    def load_x(i):
                        xtf = tmp.tile([D, N], F32, tag="xtf", name="xtf", bufs=1)
                        nc.gpsimd.dma_start(
                            xtf[:], xd[i * N:(i + 1) * N, :].rearrange("n d -> d n"))
                        xt = tmp.tile([D, N], BF16, tag="xt", name="xt")
                        nc.vector.tensor_copy(xt[:], xtf[:])
                        return xt

                    def fwd1(m1, xt):
                        # h1/u1 for feature block m1 from xt (chunk's transposed x)
                        hps = pp.tile([128, N], F32, tag="h1ps", name="hps", bufs=2)
                        nc.tensor.matmul(hps[:], w1sb[:, m1 * 128:(m1 + 1) * 128],
                                         xt[:], start=True, stop=True)
                        nc.scalar.activation(h1[:, m1], hps[:], AF.Tanh,
                                             bias=b1c[:, m1:m1 + 1])
                        nc.vector.tensor_mul(u1[:, m1], h1[:, m1], h1[:, m1])
                        nc.vector.tensor_scalar(u1[:, m1], u1[:, m1], -1.0, 1.0,
                                                ALU.mult, ALU.add)

                    xt_cur = load_x(0)
                    for m1 in range(KT):
                        fwd1(m1, xt_cur)

                    # ---- main loop over chunks ----
                    for i in range(CH):
                        # ---- phase A: fwd W2 / W3 / g2 ----
                        yvp = pp.tile([2, N], F32, tag="yvp", name="yvp", bufs=1)
                        h2t_prev = None
                        for m2 in range(KT):
                            ps = pp.tile([128, N], F32, tag="ps", name="ps", bufs=2)
                            for k in range(KT):
                                nc.tensor.matmul(
                                    ps[:], w2f[:, k, m2 * 128:(m2 + 1) * 128],
                                    h1[:, k], start=(k == 0), stop=(k == KT - 1))
                            if m2 > 0:
                                nc.tensor.matmul(yvp[:], w3sb[:, m2 - 1],
                                                 h2t_prev[:],
                                                 start=(m2 == 1), stop=False)
                            h2t = tmp.tile([128, N], BF16, tag="h2t", name="h2t",
                                           bufs=2)
                            nc.scalar.activation(h2t[:], ps[:], AF.Tanh,
                                                 bias=b2c[:, m2:m2 + 1])
                            nc.vector.tensor_mul(g2[:, m2], h2t[:], h2t[:])
                            nc.vector.tensor_scalar(
                                g2[:, m2], g2[:, m2], nw3c1[:, m2], w3c1[:, m2],
                                ALU.mult, ALU.add)
                            h2t_prev = h2t
                        nc.tensor.matmul(yvp[:], w3sb[:, KT - 1], h2t_prev[:],
                                         start=False, stop=True)
                        yvt = tmp.tile([2, N], F32, tag="yvt", name="yvt", bufs=1)
                        nc.vector.tensor_scalar(yvt[:], yvp[:], b3c[:], None,
                                                ALU.add)
                        nc.sync.dma_start(yv_d[:, i * N:(i + 1) * N], yvt[:])

                        # x for next chunk
                        if i + 1 < CH:
                            xt_cur = load_x(i + 1)

                        # ---- phase C: bwd W2^T / dVdx (+ fwd W1 of chunk i+1) ----
                        dvp = pp.tile([D, N], F32, tag="dvp", name="dvp", bufs=1)
                        g1h_prev = None
                        for m1 in range(KT):
                            gps = pp.tile([128, N], F32, tag="gps", name="gps", bufs=2)
                            korder = (list(range(KT)) if m1 < KT - 1
                                      else list(range(KT - 1, -1, -1)))
                            for j, k2 in enumerate(korder):
                                nc.tensor.matmul(
                                    gps[:], w2b[:, k2, m1 * 128:(m1 + 1) * 128],
                                    g2[:, k2], start=(j == 0), stop=(j == KT - 1))
                            if m1 > 0:
                                nc.tensor.matmul(dvp[:], w1t[:, m1 - 1],
                                                 g1h_prev[:],
                                                 start=(m1 == 1), stop=False)
                            g1h = tmp.tile([128, N], BF16, tag="g1h", name="g1h",
                                           bufs=2)
                            nc.vector.tensor_mul(g1h[:], gps[:], u1[:, m1])
                            # interleaved fwd W1 for chunk i+1 (overwrites h1/u1[m1]
                            # after their last chunk-i use)
                            if i + 1 < CH:
                                fwd1(m1, xt_cur)
                            g1h_prev = g1h
                        nc.tensor.matmul(dvp[:], w1t[:, KT - 1], g1h_prev[:],
                                         start=False, stop=True)
                        dvt = tmp.tile([D, N], F32, tag="dvt", name="dvt", bufs=1)
                        nc.vector.tensor_copy(dvt[:], dvp[:])
                        nc.sync.dma_start(dv_d[:, i * N:(i + 1) * N], dvt[:])

                # ---- final stage: batch-major per-sample math ----
                with tc.tile_pool(name="fpool", bufs=1) as fpool:
                    _final_stage(nc, tc, fpool, xd, yd, yv_d, dv_d,
                                 loss_out, part_out)

    nc.compile()
    return nc


def _final_stage(nc, tc, fpool, xd, yd, yv_d, dv_d, loss_out, part_out):
    def plane_from_row(dram_row_ap, tag):
        t = fpool.tile([128, FB], F32, tag=tag, name=tag)
        nc.sync.dma_start(t[:], dram_row_ap.rearrange("(p f) -> p f", p=128))
        return t

    ypred = plane_from_row(yv_d[0], "ypred")
    vpl = plane_from_row(yv_d[1], "vpl")
    dv0 = plane_from_row(dv_d[0], "dv0")
    dv1 = plane_from_row(dv_d[1], "dv1")
    dv2 = plane_from_row(dv_d[2], "dv2")
    dv3 = plane_from_row(dv_d[3], "dv3")
    ypl = plane_from_row(yd[:], "ypl")

    xpl = fpool.tile([128, FB, D], F32, tag="xpl", name="xpl")
    nc.sync.dma_start(xpl[:], xd.rearrange("(p f) d -> p f d", p=128))
    x2 = xpl[:, :, 1]
    x3 = xpl[:, :, 2]
    x4 = xpl[:, :, 3]

    zc = fpool.tile([128, 1], F32, tag="zc", name="zc")
    nc.vector.memset(zc[:], 0.0)

    def ftile(tag):
        return fpool.tile([128, FB], F32, tag=tag, name=tag)

    def sin_reduced(src_ap, negate, bias, tag):
        # sin(bias + (negate ? -src : src)), range-reduced mod 2pi
        w = ftile(tag + "w")
        nc.vector.tensor_scalar(w[:], src_ap, -1.0 if negate else 1.0, bias,
                                ALU.mult, ALU.add)
        t = ftile(tag + "t")
        nc.vector.tensor_scalar(t[:], w[:], INV2PI, RC, ALU.mult, ALU.add)
        r = ftile(tag + "r")
        nc.vector.tensor_scalar(r[:], t[:], RC, None, ALU.subtract)
        a = ftile(tag + "a")
        nc.vector.scalar_tensor_tensor(a[:], r[:], -TWOPI_HI, w[:], ALU.mult, ALU.add)
        y_ = ftile(tag + "y")
        nc.vector.scalar_tensor_tensor(y_[:], r[:], -TWOPI_LO, a[:], ALU.mult, ALU.add)
        o = ftile(tag + "o")
        nc.scalar.activation(o[:], y_[:], AF.Sin, bias=zc[:])
        return o

    s = sin_reduced(x3, False, 0.0, "s")
    c = sin_reduced(x3, True, HALFPI, "c")     # cos(x) = sin(pi/2 - x)

    f = ftile("f")
    nc.vector.scalar_tensor_tensor(f[:], x2, -AT, ypred[:], ALU.mult, ALU.add)

    u = ftile("u")
    nc.vector.tensor_mul(u[:], c[:], c[:])
    den = ftile("den")
    nc.vector.tensor_scalar(den[:], u[:], -C1SQ, C2C3, ALU.mult, ALU.add)
    rden = ftile("rden")
    nc.vector.reciprocal(rden[:], den[:])

    cs = ftile("cs")
    nc.vector.tensor_mul(cs[:], c[:], s[:])
    x4sq = ftile("x4sq")
    nc.vector.tensor_mul(x4sq[:], x4, x4)
    cx4 = ftile("cx4")
    nc.vector.tensor_mul(cx4[:], c[:], x4)
    sx4sq = ftile("sx4sq")
    nc.vector.tensor_mul(sx4sq[:], s[:], x4sq[:])
    csx4sq = ftile("csx4sq")
    nc.vector.tensor_mul(csx4sq[:], cs[:], x4sq[:])
    cf = ftile("cf")
    nc.vector.tensor_mul(cf[:], c[:], f[:])

    # x2p = (G*C1^2*c*s + C2*f - AR*C1*c*x4 - C1*C2*s*x4^2) / den
    p1 = ftile("p1")
    nc.vector.tensor_scalar(p1[:], f[:], C2, None, ALU.mult)
    nc.vector.scalar_tensor_tensor(p1[:], cs[:], G * C1SQ, p1[:], ALU.mult, ALU.add)
    nc.vector.scalar_tensor_tensor(p1[:], cx4[:], -AR * C1, p1[:], ALU.mult, ALU.add)
    nc.vector.scalar_tensor_tensor(p1[:], sx4sq[:], -C1 * C2, p1[:], ALU.mult, ALU.add)
    x2p = ftile("x2p")
    nc.vector.tensor_mul(x2p[:], p1[:], rden[:])

    # x4p = (G*C1*C3*s + C1*c*f - AR*C3*x4 - C1^2*c*s*x4^2) / den
    p2 = ftile("p2")
    nc.vector.tensor_scalar(p2[:], s[:], G * C1 * C3, None, ALU.mult)
    nc.vector.scalar_tensor_tensor(p2[:], cf[:], C1, p2[:], ALU.mult, ALU.add)
    nc.vector.scalar_tensor_tensor(p2[:], x4, -AR * C3, p2[:], ALU.mult, ALU.add)
    nc.vector.scalar_tensor_tensor(p2[:], csx4sq[:], -C1SQ, p2[:], ALU.mult, ALU.add)
    x4p = ftile("x4p")
    nc.vector.tensor_mul(x4p[:], p2[:], rden[:])

    # Vdot = dV . [x2, x2p, x4, x4p]
    vd = ftile("vd")
    nc.vector.tensor_mul(vd[:], dv0[:], x2)
    t_ = ftile("vt")
    nc.vector.tensor_mul(t_[:], dv1[:], x2p[:])
    nc.vector.tensor_add(vd[:], vd[:], t_[:])
    nc.vector.tensor_mul(t_[:], dv2[:], x4)
    nc.vector.tensor_add(vd[:], vd[:], t_[:])
    nc.vector.tensor_mul(t_[:], dv3[:], x4p[:])
    nc.vector.tensor_add(vd[:], vd[:], t_[:])

    # penalties: PEN*relu(-V) + PEN*relu(Vdot)
    pen = ftile("pen")
    nc.vector.tensor_scalar(pen[:], vpl[:], 0.0, -PEN, ALU.min, ALU.mult)
    pen2 = ftile("pen2")
    nc.vector.tensor_scalar(pen2[:], vd[:], 0.0, PEN, ALU.max, ALU.mult)
    nc.vector.tensor_add(pen[:], pen[:], pen2[:])
    nc.sync.dma_start(loss_out.rearrange("(p f) -> p f", p=128), pen[:])

    # partial sums for custom_loss: sum(d^2), sum((y - y_pred)^2)
    ypc = ftile("ypc")
    nc.vector.tensor_scalar(ypc[:], ypred[:], EPS, None, ALU.max)
    l1 = ftile("l1")
    nc.scalar.activation(l1[:], ypc[:], AF.Ln, bias=1.0)
    yc = ftile("yc")
    nc.vector.tensor_scalar(yc[:], ypl[:], EPS, None, ALU.max)
    l2 = ftile("l2")
    nc.scalar.activation(l2[:], yc[:], AF.Ln, bias=1.0)
    dd = ftile("dd")
    nc.vector.tensor_sub(dd[:], l1[:], l2[:])
    d2s = fpool.tile([128, 1], F32, tag="d2s", name="d2s")
    dtmp = ftile("dtmp")
    nc.scalar.activation(dtmp[:], dd[:], AF.Square, bias=zc[:], accum_out=d2s[:])
    ee = ftile("ee")
    nc.vector.tensor_sub(ee[:], ypl[:], ypred[:])
    es = fpool.tile([128, 1], F32, tag="es", name="es")
    nc.scalar.activation(dtmp[:], ee[:], AF.Square, bias=zc[:], accum_out=es[:])

    parts = fpool.tile([128, 2], F32, tag="parts", name="parts")
    nc.vector.tensor_copy(parts[:, 0:1], d2s[:])
    nc.vector.tensor_copy(parts[:, 1:2], es[:])
    nc.sync.dma_start(part_out[:, :], parts[:])


def kernel(**inputs):
    x = np.ascontiguousarray(inputs["x"], dtype=np.float32)
    y = np.ascontiguousarray(inputs["y"], dtype=np.float32)
    W1 = np.ascontiguousarray(inputs["W1"], dtype=np.float32)
    b1 = np.ascontiguousarray(inputs["b1"], dtype=np.float32)
    W2 = np.ascontiguousarray(inputs["W2"], dtype=np.float32)
    b2 = np.ascontiguousarray(inputs["b2"], dtype=np.float32)
    W3 = np.ascontiguousarray(inputs["W3"], dtype=np.float32)
    b3 = np.ascontiguousarray(inputs["b3"], dtype=np.float32)

    if "nc" not in _NC_CACHE:
        _NC_CACHE["nc"] = build()
    nc = _NC_CACHE["nc"]

    in_maps = []
    for cid in range(NCORES):
        sl = slice(cid * BC, (cid + 1) * BC)
        in_maps.append({
            "x": x[sl], "y": y[sl],
            "W1": W1, "b1": b1, "W2": W2, "b2": b2, "W3": W3, "b3": b3,
        })
    res = run_bass_kernel_spmd(nc, in_maps, list(range(NCORES)))

    loss = np.concatenate([res.results[c]["loss_pen"] for c in range(NCORES)])
    parts = np.stack([res.results[c]["partials"] for c in range(NCORES)])
    sums = parts.astype(np.float64).sum(axis=(0, 1))
    scalar = ALPHA * sums[0] / B + (1.0 - ALPHA) * sums[1] / B
    return (loss + np.float32(scalar)).astype(np.float32)


# revision 15
# speedup vs baseline: 1.2781x; 1.1007x over previous
"""Trainium2 Bass kernel for nn_CustomModel_21019569946955 (pendulum Lyapunov loss).

Data-parallel over 8 NeuronCores: each core processes B/8 = 8192 samples with
replicated MLP weights. Fully fused single-pass fp32->bf16 pipeline:

  W2 and W2^T both live in SBUF as bf16 (8 MB each), so forward and backward
  run chunk-by-chunk with no DRAM spill and no h1 recompute:

  per chunk (N=512 samples, feature-major):
    fwd:  h1 = tanh(W1^T x^T); u1 = 1-h1^2
          h2 = tanh(W2^T h1); [y_pred; V] = W3^T h2; g2 = (1-h2^2)*W3[:,1]
    bwd:  g1 = u1 * (W2 g2); dVdx = W1 g1
  final stage: batch-major pendulum ODE + penalties + partial sums for the
  scalar custom_loss (combined on host: pure data-parallel mean).
"""
import numpy as np
import concourse.bass as bass
import concourse.tile as tile
from concourse import bacc, mybir
from concourse.bass_utils import run_bass_kernel_spmd
from concourse.masks import make_identity

F32 = mybir.dt.float32
BF16 = mybir.dt.bfloat16
FP16 = mybir.dt.float16
AF = mybir.ActivationFunctionType
ALU = mybir.AluOpType

# problem constants (hardcoded from the reference)
G = 9.8
L, I_, MB, MC, AT, AR = 0.3, 2.0, 1.0, 3.0, 0.2, 0.2
C1 = L * MB            # 0.3
C2 = I_ + L * L * MB   # 2.09
C3 = MB + MC           # 4.0
PEN = 10000.0
ALPHA = 0.1
EPS = 1e-7
C1SQ = C1 * C1
C2C3 = C2 * C3

B, H, D = 65536, 2048, 4
NCORES = 8
BC = B // NCORES        # 8192 samples per core
N = 512                 # batch-chunk (moving free dim)
CH = BC // N            # 16 chunks
KT = H // 128           # 16 feature tiles
FB = BC // 128          # 64 samples per partition in the final stage

# fp32 round-to-nearest-int trick + Cody-Waite 2pi for sin/cos range reduction
RC = float(1.5 * 2 ** 23)
INV2PI = float(1.0 / (2.0 * np.pi))
TWOPI_HI = float(np.float32(2.0 * np.pi))
TWOPI_LO = float(2.0 * np.pi - np.float64(np.float32(2.0 * np.pi)))
HALFPI = float(np.pi / 2)

_NC_CACHE = {}


def build():
    nc = bacc.Bacc("TRN2", target_bir_lowering=False, debug=False)

    xd = nc.declare_dram_parameter("x", [BC, D], F32, isOutput=False)
    yd = nc.declare_dram_parameter("y", [BC], F32, isOutput=False)
    W1d = nc.declare_dram_parameter("W1", [D, H], F32, isOutput=False)
    b1d = nc.declare_dram_parameter("b1", [H], F32, isOutput=False)
    W2d = nc.declare_dram_parameter("W2", [H, H], F32, isOutput=False)
    b2d = nc.declare_dram_parameter("b2", [H], F32, isOutput=False)
    W3d = nc.declare_dram_parameter("W3", [H, 2], F32, isOutput=False)
    b3d = nc.declare_dram_parameter("b3", [2], F32, isOutput=False)

    loss_out = nc.declare_dram_parameter("loss_pen", [BC], F32, isOutput=True)
    part_out = nc.declare_dram_parameter("partials", [128, 2], F32, isOutput=True)

    with tile.TileContext(nc) as tc:
        with tc.tile_pool(name="dram", bufs=1, space="DRAM") as dpool:
            yv_d = dpool.tile([2, BC], F32, tag="yv_d", name="yv_d")
            dv_d = dpool.tile([D, BC], F32, tag="dv_d", name="dv_d")

            with tc.tile_pool(name="wpool", bufs=1) as wpool, \
                 tc.tile_pool(name="small", bufs=1) as small:

                # ---- persistent big tiles ----
                w2f = wpool.tile([128, KT, H], BF16, tag="w2f", name="w2f")
                w2b = wpool.tile([128, KT, H], BF16, tag="w2b", name="w2b")
                h1 = wpool.tile([128, KT, N], BF16, tag="h1", name="h1")
                u1 = wpool.tile([128, KT, N], BF16, tag="u1", name="u1")
                g2 = wpool.tile([128, KT, N], BF16, tag="g2", name="g2")

                # ---- small weights / constants ----
                w1sb = small.tile([D, H], BF16, tag="w1sb", name="w1sb")
                w1t = small.tile([128, KT, D], BF16, tag="w1t", name="w1t")
                w3sb = small.tile([128, KT, 2], BF16, tag="w3sb", name="w3sb")
                identb = small.tile([128, 128], BF16, tag="identb", name="identb")

                with tc.tile_pool(name="cvt", bufs=1) as cvt:
                    w1f = cvt.tile([D, H], F32, tag="w1f", name="w1f")
                    nc.sync.dma_start(w1f[:], W1d[:, :])
                    nc.vector.tensor_copy(w1sb[:], w1f[:])

                    w1tf = cvt.tile([128, KT, D], F32, tag="w1tf", name="w1tf")
                    for k in range(KT):
                        nc.gpsimd.dma_start(
                            w1tf[:, k],
                            W1d[:, k * 128:(k + 1) * 128].rearrange("d p -> p d"))
                    nc.vector.tensor_copy(w1t[:], w1tf[:])

                    w3f = cvt.tile([128, KT, 2], F32, tag="w3f", name="w3f")
                    nc.gpsimd.dma_start(w3f[:], W3d.rearrange("(k p) j -> p k j", p=128))
                    nc.vector.tensor_copy(w3sb[:], w3f[:])

                    ident_f = cvt.tile([128, 128], F32, tag="ident_f", name="ident_f")
                    make_identity(nc, ident_f[:])
                    nc.vector.tensor_copy(identb[:], ident_f[:])

                b1c = small.tile([128, KT], F32, tag="b1c", name="b1c")
                nc.gpsimd.dma_start(b1c[:], b1d.rearrange("(k p) -> p k", p=128))
                b2c = small.tile([128, KT], F32, tag="b2c", name="b2c")
                nc.gpsimd.dma_start(b2c[:], b2d.rearrange("(k p) -> p k", p=128))
                b3c = small.tile([2, 1], F32, tag="b3c", name="b3c")
                nc.gpsimd.dma_start(b3c[:], b3d.rearrange("(p o) -> p o", o=1))
                w3c1 = small.tile([128, KT, 1], F32, tag="w3c1", name="w3c1")
                nc.gpsimd.dma_start(
                    w3c1[:], W3d.rearrange("(k p) j -> p k j", p=128)[:, :, 1:2])
                nw3c1 = small.tile([128, KT, 1], F32, tag="nw3c1", name="nw3c1")
                nc.vector.tensor_scalar_mul(nw3c1[:], w3c1[:], -1.0)



                with tc.tile_pool(name="tmp", bufs=2) as tmp, \
                     tc.tile_pool(name="pp", bufs=2, space="PSUM") as pp:

                    # ---- W2 load + convert + transpose (prologue) ----
                    HQ = H // 4
                    with tc.tile_pool(name="w2cv", bufs=2) as cvp:
                        for k in range(KT):
                            for hh in range(4):
                                t = cvp.tile([128, HQ], F32, tag="w2tmp",
                                             name="w2tmp")
                                nc.sync.dma_start(
                                    t[:], W2d[k * 128:(k + 1) * 128,
                                              hh * HQ:(hh + 1) * HQ])
                                nc.vector.tensor_copy(
                                    w2f[:, k, hh * HQ:(hh + 1) * HQ], t[:])
                                # W2^T tiles: w2b[:, m, k*128..] for this quarter
                                for m in range(4 * hh, 4 * hh + 4):
                                    trp = pp.tile([128, 128], BF16, tag="gps",
                                                  name="trp", bufs=2)
                                    nc.tensor.transpose(
                                        trp[:], w2f[:, k, m * 128:(m + 1) * 128],
                                        identb[:])
                                    if m % 2 == 0:
                                        nc.vector.tensor_copy(
                                            w2b[:, m, k * 128:(k + 1) * 128], trp[:])
                                    else:
                                        nc.scalar.activation(
                                            w2b[:, m, k * 128:(k + 1) * 128], trp[:],
                                            AF.Copy, bias=0.0)

                    # ---- prologue: x chunk 0 + h1/u1 chunk 0 ----
                    def load_x(i):
                        xtf = tmp.tile([D, N], F32, tag="xtf", name="xtf", bufs=1)
                        nc.gpsimd.dma_start(
                            xtf[:], xd[i * N:(i + 1) * N, :].rearrange("n d -> d n"))
                        xt = tmp.tile([D, N], BF16, tag="xt", name="xt")
                        nc.vector.tensor_copy(xt[:], xtf[:])
                        return xt

                    def fwd1(m1, xt):
                        # h1/u1 for feature block m1 from xt (chunk's transposed x)
                        hps = pp.tile([128, N], F32, tag="h1ps", name="hps", bufs=2)
                        nc.tensor.matmul(hps[:], w1sb[:, m1 * 128:(m1 + 1) * 128],
                                         xt[:], start=True, stop=True)
                        nc.scalar.activation(h1[:, m1], hps[:], AF.Tanh,
                                             bias=b1c[:, m1:m1 + 1])
                        nc.vector.tensor_mul(u1[:, m1], h1[:, m1], h1[:, m1])
                        nc.vector.tensor_scalar(u1[:, m1], u1[:, m1], -1.0, 1.0,
                                                ALU.mult, ALU.add)

                    xt_cur = load_x(0)
                    for m1 in range(KT):
                        fwd1(m1, xt_cur)

                    # ---- main loop over chunks ----
                    for i in range(CH):
                        # ---- phase A: fwd W2 / W3 / g2 ----
                        yvp = pp.tile([2, N], F32, tag="yvp", name="yvp", bufs=1)
                        h2t_prev = None
                        for m2 in range(KT):
                            ps = pp.tile([128, N], F32, tag="ps", name="ps", bufs=2)
                            for k in range(KT):
                                nc.tensor.matmul(
                                    ps[:], w2f[:, k, m2 * 128:(m2 + 1) * 128],
                                    h1[:, k], start=(k == 0), stop=(k == KT - 1))
                            if m2 > 0:
                                nc.tensor.matmul(yvp[:], w3sb[:, m2 - 1],
                                                 h2t_prev[:],
                                                 start=(m2 == 1), stop=False)
                            h2t = tmp.tile([128, N], BF16, tag="h2t", name="h2t",
                                           bufs=2)
                            nc.scalar.activation(h2t[:], ps[:], AF.Tanh,
                                                 bias=b2c[:, m2:m2 + 1])
                            nc.vector.tensor_mul(g2[:, m2], h2t[:], h2t[:])
                            nc.vector.tensor_scalar(
                                g2[:, m2], g2[:, m2], nw3c1[:, m2], w3c1[:, m2],
                                ALU.mult, ALU.add)
                            h2t_prev = h2t
                        nc.tensor.matmul(yvp[:], w3sb[:, KT - 1], h2t_prev[:],
                                         start=False, stop=True)
                        yvt = tmp.tile([2, N], F32, tag="yvt", name="yvt", bufs=1)
                        nc.vector.tensor_scalar(yvt[:], yvp[:], b3c[:], None,
                                                ALU.add)
                        nc.sync.dma_start(yv_d[:, i * N:(i + 1) * N], yvt[:])

                        # x for next chunk
                        if i + 1 < CH:
                            xt_cur = load_x(i + 1)

                        # ---- phase C: bwd W2^T / dVdx (+ fwd W1 of chunk i+1) ----
                        dvp = pp.tile([D, N], F32, tag="dvp", name="dvp", bufs=1)
                        g1h_prev = None
                        for m1 in range(KT):
                            gps = pp.tile([128, N], F32, tag="gps", name="gps", bufs=2)
                            korder = (list(range(KT)) if m1 < KT - 1
                                      else list(range(KT - 1, -1, -1)))
                            for j, k2 in enumerate(korder):
                                nc.tensor.matmul(
                                    gps[:], w2b[:, k2, m1 * 128:(m1 + 1) * 128],
                                    g2[:, k2], start=(j == 0), stop=(j == KT - 1))
                            if m1 > 0:
                                nc.tensor.matmul(dvp[:], w1t[:, m1 - 1],
                                                 g1h_prev[:],
                                                 start=(m1 == 1), stop=False)
                            g1h = tmp.tile([128, N], BF16, tag="g1h", name="g1h",
                                           bufs=2)
                            nc.vector.tensor_mul(g1h[:], gps[:], u1[:, m1])
                            # interleaved fwd W1 for chunk i+1 (overwrites h1/u1[m1]
                            # after their last chunk-i use)
                            if i + 1 < CH:
                                fwd1(m1, xt_cur)
                            g1h_prev = g1h
                        nc.tensor.matmul(dvp[:], w1t[:, KT - 1], g1h_prev[:],
                                         start=False, stop=True)
                        dvt = tmp.tile([D, N], F32, tag="dvt", name="dvt", bufs=1)
                        nc.vector.tensor_copy(dvt[:], dvp[:])
                        nc.sync.dma_start(dv_d[:, i * N:(i + 1) * N], dvt[:])

                # ---- final stage: batch-major per-sample math ----
                with tc.tile_pool(name="fpool", bufs=1) as fpool:
                    _final_stage(nc, tc, fpool, xd, yd, yv_d, dv_d,
                                 loss_out, part_out)

    nc.compile()
    return nc


def _final_stage(nc, tc, fpool, xd, yd, yv_d, dv_d, loss_out, part_out):
    def plane_from_row(dram_row_ap, tag):
        t = fpool.tile([128, FB], F32, tag=tag, name=tag)
        nc.sync.dma_start(t[:], dram_row_ap.rearrange("(p f) -> p f", p=128))
        return t

    ypred = plane_from_row(yv_d[0], "ypred")
    vpl = plane_from_row(yv_d[1], "vpl")
    dv0 = plane_from_row(dv_d[0], "dv0")
    dv1 = plane_from_row(dv_d[1], "dv1")
    dv2 = plane_from_row(dv_d[2], "dv2")
    dv3 = plane_from_row(dv_d[3], "dv3")
    ypl = plane_from_row(yd[:], "ypl")

    xpl = fpool.tile([128, FB, D], F32, tag="xpl", name="xpl")
    nc.sync.dma_start(xpl[:], xd.rearrange("(p f) d -> p f d", p=128))
    x2 = xpl[:, :, 1]
    x3 = xpl[:, :, 2]
    x4 = xpl[:, :, 3]

    zc = fpool.tile([128, 1], F32, tag="zc", name="zc")
    nc.vector.memset(zc[:], 0.0)

    def ftile(tag):
        return fpool.tile([128, FB], F32, tag=tag, name=tag)

    def sin_reduced(src_ap, negate, bias, tag):
        # sin(bias + (negate ? -src : src)), range-reduced mod 2pi
        w = ftile(tag + "w")
        nc.vector.tensor_scalar(w[:], src_ap, -1.0 if negate else 1.0, bias,
                                ALU.mult, ALU.add)
        t = ftile(tag + "t")
        nc.vector.tensor_scalar(t[:], w[:], INV2PI, RC, ALU.mult, ALU.add)
        r = ftile(tag + "r")
        nc.vector.tensor_scalar(r[:], t[:], RC, None, ALU.subtract)
        a = ftile(tag + "a")
        nc.vector.scalar_tensor_tensor(a[:], r[:], -TWOPI_HI, w[:], ALU.mult, ALU.add)
        y_ = ftile(tag + "y")
        nc.vector.scalar_tensor_tensor(y_[:], r[:], -TWOPI_LO, a[:], ALU.mult, ALU.add)
        o = ftile(tag + "o")
        nc.scalar.activation(o[:], y_[:], AF.Sin, bias=zc[:])
        return o

    s = sin_reduced(x3, False, 0.0, "s")
    c = sin_reduced(x3, True, HALFPI, "c")     # cos(x) = sin(pi/2 - x)

    f = ftile("f")
    nc.vector.scalar_tensor_tensor(f[:], x2, -AT, ypred[:], ALU.mult, ALU.add)

    u = ftile("u")
    nc.vector.tensor_mul(u[:], c[:], c[:])
    den = ftile("den")
    nc.vector.tensor_scalar(den[:], u[:], -C1SQ, C2C3, ALU.mult, ALU.add)
    rden = ftile("rden")
    nc.vector.reciprocal(rden[:], den[:])

    cs = ftile("cs")
    nc.vector.tensor_mul(cs[:], c[:], s[:])
    x4sq = ftile("x4sq")
    nc.vector.tensor_mul(x4sq[:], x4, x4)
    cx4 = ftile("cx4")
    nc.vector.tensor_mul(cx4[:], c[:], x4)
    sx4sq = ftile("sx4sq")
    nc.vector.tensor_mul(sx4sq[:], s[:], x4sq[:])
    csx4sq = ftile("csx4sq")
    nc.vector.tensor_mul(csx4sq[:], cs[:], x4sq[:])
    cf = ftile("cf")
    nc.vector.tensor_mul(cf[:], c[:], f[:])

    # x2p = (G*C1^2*c*s + C2*f - AR*C1*c*x4 - C1*C2*s*x4^2) / den
    p1 = ftile("p1")
    nc.vector.tensor_scalar(p1[:], f[:], C2, None, ALU.mult)
    nc.vector.scalar_tensor_tensor(p1[:], cs[:], G * C1SQ, p1[:], ALU.mult, ALU.add)
    nc.vector.scalar_tensor_tensor(p1[:], cx4[:], -AR * C1, p1[:], ALU.mult, ALU.add)
    nc.vector.scalar_tensor_tensor(p1[:], sx4sq[:], -C1 * C2, p1[:], ALU.mult, ALU.add)
    x2p = ftile("x2p")
    nc.vector.tensor_mul(x2p[:], p1[:], rden[:])

    # x4p = (G*C1*C3*s + C1*c*f - AR*C3*x4 - C1^2*c*s*x4^2) / den
    p2 = ftile("p2")
    nc.vector.tensor_scalar(p2[:], s[:], G * C1 * C3, None, ALU.mult)
    nc.vector.scalar_tensor_tensor(p2[:], cf[:], C1, p2[:], ALU.mult, ALU.add)
    nc.vector.scalar_tensor_tensor(p2[:], x4, -AR * C3, p2[:], ALU.mult, ALU.add)
    nc.vector.scalar_tensor_tensor(p2[:], csx4sq[:], -C1SQ, p2[:], ALU.mult, ALU.add)
    x4p = ftile("x4p")
    nc.vector.tensor_mul(x4p[:], p2[:], rden[:])

    # Vdot = dV . [x2, x2p, x4, x4p]
    vd = ftile("vd")
    nc.vector.tensor_mul(vd[:], dv0[:], x2)
    t_ = ftile("vt")
    nc.vector.tensor_mul(t_[:], dv1[:], x2p[:])
    nc.vector.tensor_add(vd[:], vd[:], t_[:])
    nc.vector.tensor_mul(t_[:], dv2[:], x4)
    nc.vector.tensor_add(vd[:], vd[:], t_[:])
    nc.vector.tensor_mul(t_[:], dv3[:], x4p[:])
    nc.vector.tensor_add(vd[:], vd[:], t_[:])

    # penalties: PEN*relu(-V) + PEN*relu(Vdot)
    pen = ftile("pen")
    nc.vector.tensor_scalar(pen[:], vpl[:], 0.0, -PEN, ALU.min, ALU.mult)
    pen2 = ftile("pen2")
    nc.vector.tensor_scalar(pen2[:], vd[:], 0.0, PEN, ALU.max, ALU.mult)
    nc.vector.tensor_add(pen[:], pen[:], pen2[:])
    nc.sync.dma_start(loss_out.rearrange("(p f) -> p f", p=128), pen[:])

    # partial sums for custom_loss: sum(d^2), sum((y - y_pred)^2)
    ypc = ftile("ypc")
    nc.vector.tensor_scalar(ypc[:], ypred[:], EPS, None, ALU.max)
    l1 = ftile("l1")
    nc.scalar.activation(l1[:], ypc[:], AF.Ln, bias=1.0)
    yc = ftile("yc")
    nc.vector.tensor_scalar(yc[:], ypl[:], EPS, None, ALU.max)
    l2 = ftile("l2")
    nc.scalar.activation(l2[:], yc[:], AF.Ln, bias=1.0)
    dd = ftile("dd")
    nc.vector.tensor_sub(dd[:], l1[:], l2[:])
    d2s = fpool.tile([128, 1], F32, tag="d2s", name="d2s")
    dtmp = ftile("dtmp")
    nc.scalar.activation(dtmp[:], dd[:], AF.Square, bias=zc[:], accum_out=d2s[:])
    ee = ftile("ee")
    nc.vector.tensor_sub(ee[:], ypl[:], ypred[:])
    es = fpool.tile([128, 1], F32, tag="es", name="es")
    nc.scalar.activation(dtmp[:], ee[:], AF.Square, bias=zc[:], accum_out=es[:])

    parts = fpool.tile([128, 2], F32, tag="parts", name="parts")
    nc.vector.tensor_copy(parts[:, 0:1], d2s[:])
    nc.vector.tensor_copy(parts[:, 1:2], es[:])
    nc.sync.dma_start(part_out[:, :], parts[:])


def kernel(**inputs):
    x = np.ascontiguousarray(inputs["x"], dtype=np.float32)
    y = np.ascontiguousarray(inputs["y"], dtype=np.float32)
    W1 = np.ascontiguousarray(inputs["W1"], dtype=np.float32)
    b1 = np.ascontiguousarray(inputs["b1"], dtype=np.float32)
    W2 = np.ascontiguousarray(inputs["W2"], dtype=np.float32)
    b2 = np.ascontiguousarray(inputs["b2"], dtype=np.float32)
    W3 = np.ascontiguousarray(inputs["W3"], dtype=np.float32)
    b3 = np.ascontiguousarray(inputs["b3"], dtype=np.float32)

    if "nc" not in _NC_CACHE:
        _NC_CACHE["nc"] = build()
    nc = _NC_CACHE["nc"]

    in_maps = []
    for cid in range(NCORES):
        sl = slice(cid * BC, (cid + 1) * BC)
        in_maps.append({
            "x": x[sl], "y": y[sl],
            "W1": W1, "b1": b1, "W2": W2, "b2": b2, "W3": W3, "b3": b3,
        })
    res = run_bass_kernel_spmd(nc, in_maps, list(range(NCORES)))

    loss = np.concatenate([res.results[c]["loss_pen"] for c in range(NCORES)])
    parts = np.stack([res.results[c]["partials"] for c in range(NCORES)])
    sums = parts.astype(np.float64).sum(axis=(0, 1))
    scalar = ALPHA * sums[0] / B + (1.0 - ALPHA) * sums[1] / B
    return (loss + np.float32(scalar)).astype(np.float32)
